# revision 48
# baseline (speedup 1.0000x reference)
"""2-layer GCN (PyG GCNConv semantics) as a Bass/Tile kernel for TRN2.

Math (per GCNConv layer, self-loops added, deg from dst in-degree + 1):
  out[d] = b + sum_{e: dst[e]=d} w[e] * t[src[e]]      with w[e] = rsqrt(deg[src]*deg[dst])
  where t = x        (layer 1: aggregate first, then @W1 — W commutes with aggregation)
        t = y1 @ W2  (layer 2: transform first)
  self-loop appears as an ordinary edge (i,i) with w = 1/deg[i].

Device mapping per core (nodes chunked across cores, edges bucketed by dst super):
  - supers of DW=512 dst columns; edges bucketed per (super, quarter) where a
    quarter is a <=32768-row slice of the gather table (int16 index limit),
    each bucket padded to a multiple of 128 (sizes are max over cores: one
    NEFF runs SPMD on all 8 cores with per-core index/weight data).
  - dma_gather (SWDGE) fetches 256B bf16 rows; calls are grouped over GSUP
    supers x 1 quarter (the Q7 desc-gen rate of ~8ns/row is the floor).
  - scatter-add via one-hot matmul: S[e, d] = w[e] * (dst_local[e] == d),
    built on DVE with f32 inputs and bf16 output (the one shape that stays
    off the shared DVE/GpSimd SBUF port pair — other dtype combos engage
    2-port perf mode and starve the Q7 descriptor generator);
    psum[f, d] += Msg^T @ S. Self-loops via dinv-scaled identity matmuls.
  - layer-1 t output is split at super SPLIT_SP: the first part is
    AllGathered mid-layer-1 so the exchange overlaps compute, the rest at
    the end; layer 2 gathers from the two exchanged tables.
  - log_softmax is batched over all tiles at the end (one Exp + one Ln).
"""

import math
import sys

import numpy as np

sys.path.insert(0, "/opt/trn_rl_repo")

import concourse.bass as bass
import concourse.bacc as bacc
import concourse.mybir as mybir
import concourse.tile as tile
from concourse.masks import make_identity

F32 = mybir.dt.float32
BF16 = mybir.dt.bfloat16
I16 = mybir.dt.int16
I32 = mybir.dt.int32

P = 128
QS = 32768  # int16-indexable rows per gather table slice
NEG_SLOPE = 0.01
TPS = 4        # dst tiles per super (DW = TPS*128)
GSUP = 3       # supers per grouped gather call
SPLIT_SP = 13  # supers 0..SPLIT_SP-1 exchange early (rows < SPLIT_SP*512)
AG_A_AT_GG = 5 # issue the early AllGather after this gather group's calls

F_IN, H1, H2, N_CLS = 128, 180, 120, 16


class Meta:
    pass


# ---------------------------------------------------------------- host prep

def _bucketize(src, dst, w_e, q_of, idx_of, n_cores, NS, NQ, chunk, supers):
    """Bucket edges by (core, super, quarter); lay out slot stream grouped
    for gather calls (gg -> q -> sp). Returns layer struct + per-core data."""
    core_of = dst // chunk
    sup_of = (dst % chunk) // (TPS * P)

    counts = np.zeros((n_cores, NS, NQ), dtype=np.int64)
    np.add.at(counts, (core_of, sup_of, q_of), 1)
    mx = counts.max(axis=0)
    slots_sq = ((mx + P - 1) // P).astype(np.int64)

    NG = math.ceil(NS / GSUP)
    slot0 = np.zeros((NS, NQ), dtype=np.int64)
    ggroups = []
    off = 0
    for g in range(NG):
        sps = list(range(g * GSUP, min((g + 1) * GSUP, NS)))
        qg = []
        for q in range(NQ):
            g0 = off
            for sp in sps:
                slot0[sp, q] = off
                off += int(slots_sq[sp, q])
            if off > g0:
                qg.append((q, g0, off - g0))
        ggroups.append(dict(sps=sps, qgroups=qg))
    total_slots = off

    spgroups = []
    for sp in range(NS):
        spgroups.append([
            (q, int(slot0[sp, q]), int(slots_sq[sp, q]))
            for q in range(NQ)
            if slots_sq[sp, q] > 0
        ])

    order = np.lexsort((src, q_of, sup_of, core_of))
    s_s = idx_of[order]
    d_s = dst[order]
    w_s = w_e[order]
    keys = ((core_of * NS + sup_of) * NQ + q_of)[order]
    bucket_lo = np.searchsorted(keys, np.arange(n_cores * NS * NQ), side="left")
    bucket_hi = np.searchsorted(keys, np.arange(n_cores * NS * NQ), side="right")

    import ml_dtypes

    bf = ml_dtypes.bfloat16
    per_core = []
    for k in range(n_cores):
        gflat = np.zeros(total_slots * P, dtype=np.int16)
        dflat = np.full(total_slots * P, 999.0, dtype=np.float64)
        wflat = np.zeros(total_slots * P, dtype=np.float32)
        for sp in range(NS):
            for (q, g0, nsl) in spgroups[sp]:
                b = (k * NS + sp) * NQ + q
                i0, i1 = bucket_lo[b], bucket_hi[b]
                n = i1 - i0
                if n == 0:
                    continue
                pos = g0 * P
                gflat[pos : pos + n] = s_s[i0:i1].astype(np.int16)
                dflat[pos : pos + n] = d_s[i0:i1] % chunk - supers[sp]["col0"]
                wflat[pos : pos + n] = w_s[i0:i1]
        per_core.append(dict(
            gidx=np.tile(gflat.reshape(-1, 16).T, (8, 1)),
            dstloc=dflat.reshape(-1, P).T.astype(np.float32).copy(),
            wv=wflat.reshape(-1, P).T.copy(),
        ))

    layer = dict(ggroups=ggroups, spgroups=spgroups, total_slots=total_slots)
    return layer, per_core


def prep(edge_index, n_nodes, n_cores):
    src = np.asarray(edge_index[0], dtype=np.int64)
    dst = np.asarray(edge_index[1], dtype=np.int64)
    deg = np.bincount(dst, minlength=n_nodes) + 1
    w_e = (1.0 / np.sqrt(deg[src] * deg[dst])).astype(np.float32)

    assert n_nodes % n_cores == 0
    chunk = n_nodes // n_cores
    NT = math.ceil(chunk / P)
    NS = math.ceil(NT / TPS)
    DW = TPS * P

    supers = []
    for sp in range(NS):
        t0 = sp * TPS
        tiles = [(t, min(P, chunk - t * P)) for t in range(t0, min(t0 + TPS, NT))]
        supers.append(dict(sp=sp, tiles=tiles, col0=t0 * P))

    # layer 1: table = x rows (node ids), quarters of 32768
    NQ1 = math.ceil(n_nodes / QS)
    q1 = src // QS
    i1 = src - q1 * QS
    L1, pc1 = _bucketize(src, dst, w_e, q1, i1, n_cores, NS, NQ1, chunk, supers)
    L1["qbounds"] = [("X", q * QS, min(n_nodes, (q + 1) * QS)) for q in range(NQ1)]

    # layer 2: table = exchanged t rows, split into A (local row < SA) and B
    SA = SPLIT_SP * DW
    SB = chunk - SA
    rows_a = n_cores * SA
    rows_b = n_cores * SB
    c_of = src // chunk
    r_of = src % chunk
    in_a = r_of < SA
    row2 = np.where(in_a, c_of * SA + r_of, c_of * SB + (r_of - SA))
    nqa = math.ceil(rows_a / QS)
    nqb = math.ceil(rows_b / QS)
    q2 = np.where(in_a, row2 // QS, nqa + row2 // QS)
    i2 = row2 % QS
    NQ2 = nqa + nqb
    L2, pc2 = _bucketize(src, dst, w_e, q2, i2, n_cores, NS, NQ2, chunk, supers)
    L2["qbounds"] = [("A", q * QS, min(rows_a, (q + 1) * QS)) for q in range(nqa)] + [
        ("B", q * QS, min(rows_b, (q + 1) * QS)) for q in range(nqb)
    ]

    m = Meta()
    m.n_nodes = n_nodes
    m.n_cores = n_cores
    m.chunk = chunk
    m.NT = NT
    m.NS = NS
    m.dwidth = DW
    m.SA = SA
    m.SB = SB
    m.supers = supers
    m.L = [L1, L2]

    per_core = []
    for k in range(n_cores):
        gidx = np.concatenate([pc1[k]["gidx"], pc2[k]["gidx"]], axis=1)
        dstloc = np.concatenate([pc1[k]["dstloc"], pc2[k]["dstloc"]], axis=1)
        wv = np.concatenate([pc1[k]["wv"], pc2[k]["wv"]], axis=1)
        dn = np.ones(NT * P, dtype=np.float32)
        dn[:chunk] = deg[k * chunk : (k + 1) * chunk]
        degn = dn.reshape(NT, P).T.copy()
        per_core.append(dict(gidx=gidx, dstloc=dstloc, wv=wv, degn=degn))
    m.slots1 = L1["total_slots"]
    m.total_slots = L1["total_slots"] + L2["total_slots"]
    return m, per_core


# ---------------------------------------------------------------- kernel build

def build(m: Meta):
    nc = bacc.Bacc(trn_type="TRN2", num_devices=m.n_cores, target_bir_lowering=False)
    chunk, NT, NS, DW = m.chunk, m.NT, m.NS, m.dwidth

    xb_d = nc.dram_tensor("xb", [m.n_nodes, P], BF16, kind="ExternalInput")
    xown_d = nc.dram_tensor("xown", [chunk, P], BF16, kind="ExternalInput")
    w1_d = nc.dram_tensor("w1b", [F_IN, H1], BF16, kind="ExternalInput")
    w2a_d = nc.dram_tensor("w2ab", [P, H2], BF16, kind="ExternalInput")
    w2b_d = nc.dram_tensor("w2bb", [H1 - P, H2], BF16, kind="ExternalInput")
    wl_d = nc.dram_tensor("wlb", [H2, N_CLS], BF16, kind="ExternalInput")
    blx_d = nc.dram_tensor("blx4b", [1, N_CLS], BF16, kind="ExternalInput")
    b1a_d = nc.dram_tensor("b1a", [P, 1], F32, kind="ExternalInput")
    b1b_d = nc.dram_tensor("b1b", [H1 - P, 1], F32, kind="ExternalInput")
    b2_d = nc.dram_tensor("b2", [H2, 1], F32, kind="ExternalInput")
    gidx_d = nc.dram_tensor("gidx", [P, m.total_slots * 8], I16, kind="ExternalInput")
    dst_d = nc.dram_tensor("dstloc", [P, m.total_slots], F32, kind="ExternalInput")
    wv_d = nc.dram_tensor("wv", [P, m.total_slots], F32, kind="ExternalInput")
    degn_d = nc.dram_tensor("degn", [P, m.NT], F32, kind="ExternalInput")
    out_d = nc.dram_tensor("out", [chunk, N_CLS], F32, kind="ExternalOutput")

    tchA_d = nc.dram_tensor("tchA", [m.SA, P], BF16, kind="Internal")
    tchB_d = nc.dram_tensor("tchB", [m.SB, P], BF16, kind="Internal")
    tfA_d = nc.dram_tensor(
        "tfA", [m.n_cores * m.SA, P], BF16, kind="Internal", addr_space="Shared"
    )
    tfB_d = nc.dram_tensor(
        "tfB", [m.n_cores * m.SB, P], BF16, kind="Internal", addr_space="Shared"
    )

    def own_rows_l1(r0, r1):
        return xown_d[r0:r1, :]

    def own_rows_l2(r0, r1):
        if r1 <= m.SA:
            return tchA_d[r0:r1, :]
        assert r0 >= m.SA
        return tchB_d[r0 - m.SA : r1 - m.SA, :]

    from contextlib import ExitStack

    with tile.TileContext(nc) as tc, ExitStack() as ctx:
        cpool = ctx.enter_context(tc.tile_pool(name="consts", bufs=1))
        mpool = ctx.enter_context(tc.tile_pool(name="msg", bufs=3))
        spool = ctx.enter_context(tc.tile_pool(name="onehot", bufs=8))
        wkpool = ctx.enter_context(tc.tile_pool(name="work", bufs=2))
        softpool = ctx.enter_context(tc.tile_pool(name="soft", bufs=1))
        scat_pp = ctx.enter_context(tc.tile_pool(name="scat", bufs=2, space="PSUM"))
        y1a_pp = ctx.enter_context(tc.tile_pool(name="y1aps", bufs=2, space="PSUM"))
        y1b_pp = ctx.enter_context(tc.tile_pool(name="y1bps", bufs=1, space="PSUM"))
        t_pp = ctx.enter_context(tc.tile_pool(name="tps", bufs=1, space="PSUM"))
        log_pp = ctx.enter_context(tc.tile_pool(name="logps", bufs=1, space="PSUM"))

        # ---- constants / resident tiles
        w1_s = cpool.tile([F_IN, H1], BF16)
        w2a_s = cpool.tile([P, H2], BF16)
        w2b_s = cpool.tile([H1 - P, H2], BF16)
        wl_s = cpool.tile([H2, N_CLS], BF16)
        blx_s = cpool.tile([1, N_CLS], BF16)
        ones_s = cpool.tile([1, P], BF16)
        b1a_s = cpool.tile([P, 1], F32)
        b1b_s = cpool.tile([H1 - P, 1], F32)
        b2_s = cpool.tile([H2, 1], F32)
        gidx_s = cpool.tile([P, m.total_slots * 8], I16)
        dst_s = cpool.tile([P, m.total_slots], F32)
        wv_s = cpool.tile([P, m.total_slots], F32)
        negw_s = cpool.tile([P, m.total_slots], F32)
        iota_i = cpool.tile([P, DW], I32)
        iota_f = cpool.tile([P, DW], F32)
        ident_f = cpool.tile([P, P], F32)
        identw_f = cpool.tile([P, TPS, DW], F32)
        identw_b = cpool.tile([P, TPS, DW], BF16)
        degn_s = cpool.tile([P, m.NT], F32)
        dinvn_s = cpool.tile([P, m.NT], F32)
        lgall_s = cpool.tile([P, NT, N_CLS], F32)

        nc.sync.dma_start(w1_s[:], w1_d[:])
        nc.sync.dma_start(w2a_s[:], w2a_d[:])
        nc.sync.dma_start(w2b_s[:], w2b_d[:])
        nc.sync.dma_start(wl_s[:], wl_d[:])
        nc.sync.dma_start(blx_s[:], blx_d[:])
        nc.sync.dma_start(b1a_s[:], b1a_d[:])
        nc.sync.dma_start(b1b_s[:], b1b_d[:])
        nc.sync.dma_start(b2_s[:], b2_d[:])
        nc.sync.dma_start(gidx_s[:], gidx_d[:])
        nc.sync.dma_start(dst_s[:], dst_d[:])
        nc.sync.dma_start(wv_s[:], wv_d[:])
        nc.sync.dma_start(degn_s[:], degn_d[:])

        nc.vector.reciprocal(dinvn_s[:], degn_s[:])
        nc.vector.tensor_scalar_mul(negw_s[:], wv_s[:], -1.0)
        make_identity(nc, ident_f[:])
        nc.vector.memset(identw_f[:], 0)
        for ti in range(TPS):
            nc.vector.tensor_copy(identw_f[:, ti, ti * P : (ti + 1) * P], ident_f[:])
        nc.scalar.activation(
            identw_b[:], identw_f[:], mybir.ActivationFunctionType.Copy
        )
        nc.gpsimd.iota(iota_i[:], [[1, DW]], channel_multiplier=0)
        nc.vector.tensor_copy(iota_f[:], iota_i[:])
        nc.vector.memset(ones_s[:], 1.0)

        Prelu = mybir.ActivationFunctionType.Prelu
        Copy = mybir.ActivationFunctionType.Copy

        def exchange_a():
            if m.n_cores > 1:
                nc.gpsimd.collective_compute(
                    "AllGather", mybir.AluOpType.bypass,
                    replica_groups=[list(range(m.n_cores))],
                    ins=[tchA_d[:]], outs=[tfA_d[:]],
                )
            else:
                nc.sync.dma_start(tfA_d[:], tchA_d[:])

        def exchange_b():
            if m.n_cores > 1:
                nc.gpsimd.collective_compute(
                    "AllGather", mybir.AluOpType.bypass,
                    replica_groups=[list(range(m.n_cores))],
                    ins=[tchB_d[:]], outs=[tfB_d[:]],
                )
            else:
                nc.sync.dma_start(tfB_d[:], tchB_d[:])

        def layer(L, slot_off, table_aps, feat, epilogue, own_rows, post_gg=None):
            spgroups = L["spgroups"]
            for ggi, gg in enumerate(L["ggroups"]):
                msgs = {}
                for (q, g0, gn) in gg["qgroups"]:
                    ga = g0 + slot_off
                    mt = mpool.tile([P, gn, P], BF16, tag=f"msg{q % 4}")
                    nc.gpsimd.dma_gather(
                        out_ap=mt[:],
                        in_ap=table_aps[q],
                        idxs_ap=gidx_s[:, ga * 8 : (ga + gn) * 8],
                        num_idxs=gn * P,
                        num_idxs_reg=gn * P,
                        elem_size=P,
                        single_packet=False,
                    )
                    msgs[q] = (mt, g0)
                if post_gg is not None:
                    post_gg(ggi)
                for sp in gg["sps"]:
                    spm = m.supers[sp]
                    ntl = len(spm["tiles"])
                    ncols = ntl * P
                    scat = scat_pp.tile([P, DW], F32, tag="scat")
                    xt4 = wkpool.tile([P, TPS, P], BF16, tag="xt4")
                    nfull = sum(1 for (_, rows) in spm["tiles"] if rows == P)
                    t0 = spm["tiles"][0][0]
                    if nfull:
                        nc.sync.dma_start(
                            xt4[:, 0:nfull, :],
                            own_rows(t0 * P, (t0 + nfull) * P).rearrange(
                                "(a b) c -> b a c", b=P
                            ),
                        )
                    if nfull < ntl:
                        lt, lrows = spm["tiles"][nfull]
                        nc.sync.dma_start(
                            xt4[:lrows, nfull, :], own_rows(lt * P, lt * P + lrows)
                        )
                    for ti, (t, rows) in enumerate(spm["tiles"]):
                        dwt = spool.tile([P, DW], BF16, tag="S")
                        nc.scalar.activation(
                            dwt[:, :ncols], identw_b[:, ti, :ncols], Copy,
                            scale=dinvn_s[:, t : t + 1],
                        )
                        nc.tensor.matmul(
                            out=scat[:feat, :ncols],
                            lhsT=xt4[:rows, ti, :feat],
                            rhs=dwt[:rows, :ncols],
                            start=(ti == 0),
                            stop=False,
                        )
                    groups = spgroups[sp]
                    for gi, (q, g0, nsl) in enumerate(groups):
                        mt, mg0 = msgs[q]
                        for si in range(nsl):
                            g = g0 + si
                            ga = g + slot_off
                            # one-hot S[e,c] = w[e]*(c == d[e]); work is split
                            # between DVE (f32-in/bf16-out stt — the shape
                            # that avoids the shared DVE/GpSimd port pair)
                            # and the otherwise-idle ACT engine via
                            # sq=(c-d)^2 then Relu(w - w*sq).
                            S = spool.tile([P, DW], BF16, tag="S")
                            if g % 2 == 0:
                                nc.vector.scalar_tensor_tensor(
                                    out=S[:, :ncols],
                                    in0=iota_f[:, :ncols],
                                    scalar=dst_s[:, ga : ga + 1],
                                    in1=wv_s[:, ga : ga + 1].to_broadcast(
                                        [P, ncols]
                                    ),
                                    op0=mybir.AluOpType.is_equal,
                                    op1=mybir.AluOpType.mult,
                                )
                            else:
                                sq = spool.tile([P, DW], F32, tag="sq")
                                nc.scalar.activation(
                                    sq[:, :ncols], iota_f[:, :ncols],
                                    mybir.ActivationFunctionType.Square,
                                    bias=dst_s[:, ga : ga + 1], scale=-1.0,
                                )
                                nc.scalar.activation(
                                    S[:, :ncols], sq[:, :ncols],
                                    mybir.ActivationFunctionType.Relu,
                                    bias=wv_s[:, ga : ga + 1],
                                    scale=negw_s[:, ga : ga + 1],
                                )
                            nc.tensor.matmul(
                                out=scat[:feat, :ncols],
                                lhsT=mt[:, g - mg0, :feat],
                                rhs=S[:, :ncols],
                                start=False,
                                stop=(gi == len(groups) - 1 and si == nsl - 1),
                            )
                    epilogue(spm, scat)

        def write_t(t0row, nrows, t_sb, part_rows=None):
            """DMA t_sb [[128, k, 128]] to the A/B-split tchunk rows."""
            segs = []  # (dst tensor, dst row0, tile index range)
            if t0row < m.SA:
                segs.append((tchA_d, t0row, 0, min(nrows, m.SA - t0row)))
            if t0row + nrows > m.SA:
                b0 = max(t0row, m.SA)
                segs.append((tchB_d, b0 - m.SA, (b0 - t0row) // P, t0row + nrows - b0))
            for (dst, r0, ti0, rn) in segs:
                ntf = rn // P
                if ntf:
                    nc.sync.dma_start(
                        dst[r0 : r0 + ntf * P, :].rearrange("(a b) c -> b a c", b=P),
                        t_sb[:, ti0 : ti0 + ntf, :],
                    )
                rem = rn - ntf * P
                if rem:
                    nc.sync.dma_start(
                        dst[r0 + ntf * P : r0 + rn, :], t_sb[:rem, ti0 + ntf, :]
                    )

        def l1_epilogue(spm, scat):
            ntl = len(spm["tiles"])
            ncols = ntl * P
            h1b = wkpool.tile([P, DW], BF16, tag="h1b")
            nc.scalar.activation(h1b[:, :ncols], scat[:, :ncols], Copy)
            y1aps = y1a_pp.tile([P, DW], F32, tag="y1aps")
            y1bps = y1b_pp.tile([H1 - P, DW], F32, tag="y1bps")
            nc.tensor.matmul(
                out=y1aps[:, :ncols], lhsT=w1_s[:, 0:P], rhs=h1b[:, :ncols],
                start=True, stop=True,
            )
            nc.tensor.matmul(
                out=y1bps[:, :ncols], lhsT=w1_s[:, P:H1], rhs=h1b[:, :ncols],
                start=True, stop=True,
            )
            y1ab = wkpool.tile([P, DW], BF16, tag="y1ab")
            y1bb = wkpool.tile([H1 - P, DW], BF16, tag="y1bb")
            nc.scalar.activation(
                y1ab[:, :ncols], y1aps[:, :ncols], Prelu,
                bias=b1a_s[:, 0:1], scale=1.0, alpha=NEG_SLOPE,
            )
            nc.scalar.activation(
                y1bb[:, :ncols], y1bps[:, :ncols], Prelu,
                bias=b1b_s[:, 0:1], scale=1.0, alpha=NEG_SLOPE,
            )
            tps = t_pp.tile([P, TPS, H2], F32, tag="tps")
            for ti, (t, rows) in enumerate(spm["tiles"]):
                nc.tensor.matmul(
                    out=tps[:, ti, :], lhsT=y1ab[:, ti * P : (ti + 1) * P],
                    rhs=w2a_s[:], start=True, stop=False,
                )
                nc.tensor.matmul(
                    out=tps[:, ti, :], lhsT=y1bb[:, ti * P : (ti + 1) * P],
                    rhs=w2b_s[:], start=False, stop=True,
                )
            t_sb = wkpool.tile([P, TPS, P], BF16, tag="t_sb")
            nc.scalar.activation(t_sb[:, 0:ntl, 0:H2], tps[:, 0:ntl, :], Copy)
            t0 = spm["tiles"][0][0]
            nrows = sum(rows for (_, rows) in spm["tiles"])
            write_t(t0 * P, nrows, t_sb)

        def l2_epilogue(spm, scat):
            ntl = len(spm["tiles"])
            ncols = ntl * P
            y2b = wkpool.tile([H2, DW], BF16, tag="y2b")
            nc.scalar.activation(
                y2b[:, :ncols], scat[:H2, :ncols], Prelu,
                bias=b2_s[:, 0:1], scale=1.0, alpha=NEG_SLOPE,
            )
            lg = log_pp.tile([P, TPS, N_CLS], F32, tag="lg")
            for ti, (t, rows) in enumerate(spm["tiles"]):
                nc.tensor.matmul(
                    out=lg[:, ti, :], lhsT=y2b[:, ti * P : (ti + 1) * P],
                    rhs=wl_s[:], start=True, stop=False,
                )
                nc.tensor.matmul(
                    out=lg[:, ti, :], lhsT=ones_s[:],
                    rhs=blx_s[:], start=False, stop=True,
                )
            t0 = spm["tiles"][0][0]
            nc.scalar.activation(lgall_s[:, t0 : t0 + ntl, :], lg[:, 0:ntl, :], Copy)

        # ---- layer 1 (aggregate raw x, transform to t); AG_A issued mid-layer
        x_q = [xb_d[lo:hi, :] for (_, lo, hi) in m.L[0]["qbounds"]]

        def l1_post_gg(ggi):
            if ggi == AG_A_AT_GG:
                exchange_a()

        with nc.named_scope("layer1"):
            layer(m.L[0], 0, x_q, F_IN, l1_epilogue, own_rows_l1,
                  post_gg=l1_post_gg if len(m.L[0]["ggroups"]) > AG_A_AT_GG + 1
                  else None)
            if len(m.L[0]["ggroups"]) <= AG_A_AT_GG + 1:
                exchange_a()

        with nc.named_scope("exchange"):
            exchange_b()

        # ---- layer 2 + batched log_softmax
        t_q = []
        for (sp_, lo, hi) in m.L[1]["qbounds"]:
            t_q.append((tfA_d if sp_ == "A" else tfB_d)[lo:hi, :])
        with nc.named_scope("layer2"):
            layer(m.L[1], m.slots1, t_q, H2, l2_epilogue, own_rows_l2)

            Exp = mybir.ActivationFunctionType.Exp
            Ln = mybir.ActivationFunctionType.Ln
            negm = softpool.tile([P, NT, 1], F32, tag="negm")
            nc.vector.tensor_reduce(
                negm[:], lgall_s[:], mybir.AxisListType.X,
                mybir.AluOpType.max, negate=True,
            )
            # in-place: lgall becomes (logits - max); final result reuses ex
            nc.vector.tensor_tensor(
                out=lgall_s[:], in0=lgall_s[:],
                in1=negm[:].to_broadcast([P, NT, N_CLS]),
                op=mybir.AluOpType.add,
            )
            ex = softpool.tile([P, NT, N_CLS], F32, tag="ex")
            nc.scalar.activation(ex[:], lgall_s[:], Exp)
            ssum = softpool.tile([P, NT, 1], F32, tag="ssum")
            nc.vector.tensor_reduce(
                ssum[:], ex[:], mybir.AxisListType.X, mybir.AluOpType.add
            )
            lns = softpool.tile([P, NT, 1], F32, tag="lns")
            nc.scalar.activation(lns[:], ssum[:], Ln)
            osb = ex
            nc.vector.tensor_tensor(
                out=osb[:], in0=lgall_s[:],
                in1=lns[:].to_broadcast([P, NT, N_CLS]),
                op=mybir.AluOpType.subtract,
            )
            nfull_t = chunk // P
            nc.sync.dma_start(
                out_d[0 : nfull_t * P, :].rearrange("(a b) c -> b a c", b=P),
                osb[:, 0:nfull_t, :],
            )
            rem = chunk - nfull_t * P
            if rem:
                nc.sync.dma_start(
                    out_d[nfull_t * P : chunk, :], osb[:rem, nfull_t, :]
                )

    nc.compile()
    return nc


# ---------------------------------------------------------------- entry point

N_NODES = 100000
N_EDGES = 800000
N_CORES = 8

TRACE = False
LAST_EXEC_NS = None
LAST_RESULTS = None


def kernel(x, W1, b1, W2, b2, Wl, bl, edge_index):
    """Full-input GCN kernel: shards across 8 NeuronCores internally."""
    global LAST_EXEC_NS, LAST_RESULTS
    import ml_dtypes
    from concourse import bass_utils

    bf = ml_dtypes.bfloat16
    x = np.ascontiguousarray(np.asarray(x, dtype=np.float32))
    W1 = np.asarray(W1, dtype=np.float32)
    b1 = np.asarray(b1, dtype=np.float32).reshape(-1, 1)
    W2 = np.asarray(W2, dtype=np.float32)
    b2 = np.asarray(b2, dtype=np.float32).reshape(-1, 1)
    Wl = np.asarray(Wl, dtype=np.float32)
    bl = np.asarray(bl, dtype=np.float32).reshape(1, -1)
    edge_index = np.asarray(edge_index)

    n_nodes = x.shape[0]
    meta, per_core = prep(edge_index, n_nodes, n_cores=N_CORES)
    nc = build(meta)

    chunk = n_nodes // N_CORES
    xb = x.astype(bf)
    shared = dict(
        xb=xb,
        w1b=W1.astype(bf),
        w2ab=W2[:P].astype(bf),
        w2bb=W2[P:].astype(bf),
        wlb=Wl.astype(bf),
        blx4b=bl.astype(bf),
        b1a=b1[:P],
        b1b=b1[P:],
        b2=b2,
    )
    in_maps = [
        {**shared, "gidx": pc["gidx"], "dstloc": pc["dstloc"],
         "wv": pc["wv"], "degn": pc["degn"],
         "xown": xb[k * chunk : (k + 1) * chunk]}
        for k, pc in enumerate(per_core)
    ]
    res = bass_utils.run_bass_kernel_spmd(
        nc, in_maps, core_ids=list(range(N_CORES)), trace=TRACE
    )
    LAST_EXEC_NS = res.exec_time_ns
    LAST_RESULTS = res
    return np.concatenate([r["out"] for r in res.results], axis=0)


# revision 50
# speedup vs baseline: 1.1606x; 1.1606x over previous
"""2-layer GCN (PyG GCNConv semantics) as a Bass/Tile kernel for TRN2.

Math (per GCNConv layer, self-loops added, deg from dst in-degree + 1):
  out[d] = b + sum_{e: dst[e]=d} w[e] * t[src[e]]      with w[e] = rsqrt(deg[src]*deg[dst])
  where t = x        (layer 1: aggregate first, then @W1 — W commutes with aggregation)
        t = y1 @ W2  (layer 2: transform first)
  self-loop appears as an ordinary edge (i,i) with w = 1/deg[i].

Device mapping per core (nodes chunked across cores, edges bucketed by dst super):
  - supers of DW=512 dst columns; edges bucketed per (super, quarter) where a
    quarter is a <=32768-row slice of the gather table (int16 index limit),
    each bucket padded to a multiple of 128 (sizes are max over cores: one
    NEFF runs SPMD on all 8 cores with per-core index/weight data).
  - dma_gather (SWDGE) fetches 256B bf16 rows; calls are grouped over GSUP
    supers x 1 quarter (the Q7 desc-gen rate of ~8ns/row is the floor).
  - scatter-add via one-hot matmul: S[e, d] = w[e] * (dst_local[e] == d),
    built on DVE with f32 inputs and bf16 output (the one shape that stays
    off the shared DVE/GpSimd SBUF port pair — other dtype combos engage
    2-port perf mode and starve the Q7 descriptor generator);
    psum[f, d] += Msg^T @ S. Self-loops via dinv-scaled identity matmuls.
  - layer-1 t output is split at super SPLIT_SP: the first part is
    AllGathered mid-layer-1 so the exchange overlaps compute, the rest at
    the end; layer 2 gathers from the two exchanged tables.
  - log_softmax is batched over all tiles at the end (one Exp + one Ln).
"""

import math
import sys

import numpy as np

sys.path.insert(0, "/opt/trn_rl_repo")

import concourse.bass as bass
import concourse.bacc as bacc
import concourse.mybir as mybir
import concourse.tile as tile
from concourse.masks import make_identity

F32 = mybir.dt.float32
BF16 = mybir.dt.bfloat16
I16 = mybir.dt.int16
I32 = mybir.dt.int32

P = 128
QS = 32768  # int16-indexable rows per gather table slice
NEG_SLOPE = 0.01
TPS = 4        # dst tiles per super (DW = TPS*128)
GSUP = 3       # supers per grouped gather call
SPLIT_SP = 13  # supers 0..SPLIT_SP-1 exchange early (rows < SPLIT_SP*512)
AG_A_AT_GG = 5 # issue the early AllGather after this gather group's calls

F_IN, H1, H2, N_CLS = 128, 180, 120, 16


class Meta:
    pass


# ---------------------------------------------------------------- host prep

def _bucketize(src, dst, w_e, q_of, idx_of, n_cores, NS, NQ, chunk, supers):
    """Bucket edges by (core, super, quarter); lay out slot stream grouped
    for gather calls (gg -> q -> sp). Returns layer struct + per-core data."""
    core_of = dst // chunk
    sup_of = (dst % chunk) // (TPS * P)

    counts = np.zeros((n_cores, NS, NQ), dtype=np.int64)
    np.add.at(counts, (core_of, sup_of, q_of), 1)
    mx = counts.max(axis=0)
    slots_sq = ((mx + P - 1) // P).astype(np.int64)

    NG = math.ceil(NS / GSUP)
    slot0 = np.zeros((NS, NQ), dtype=np.int64)
    ggroups = []
    off = 0
    for g in range(NG):
        sps = list(range(g * GSUP, min((g + 1) * GSUP, NS)))
        qg = []
        for q in range(NQ):
            g0 = off
            for sp in sps:
                slot0[sp, q] = off
                off += int(slots_sq[sp, q])
            if off > g0:
                qg.append((q, g0, off - g0))
        ggroups.append(dict(sps=sps, qgroups=qg))
    total_slots = off

    spgroups = []
    for sp in range(NS):
        spgroups.append([
            (q, int(slot0[sp, q]), int(slots_sq[sp, q]))
            for q in range(NQ)
            if slots_sq[sp, q] > 0
        ])

    order = np.lexsort((src, q_of, sup_of, core_of))
    s_s = idx_of[order]
    d_s = dst[order]
    w_s = w_e[order]
    keys = ((core_of * NS + sup_of) * NQ + q_of)[order]
    bucket_lo = np.searchsorted(keys, np.arange(n_cores * NS * NQ), side="left")
    bucket_hi = np.searchsorted(keys, np.arange(n_cores * NS * NQ), side="right")

    import ml_dtypes

    bf = ml_dtypes.bfloat16
    per_core = []
    for k in range(n_cores):
        gflat = np.zeros(total_slots * P, dtype=np.int16)
        dflat = np.full(total_slots * P, 999.0, dtype=np.float64)
        wflat = np.zeros(total_slots * P, dtype=np.float32)
        for sp in range(NS):
            for (q, g0, nsl) in spgroups[sp]:
                b = (k * NS + sp) * NQ + q
                i0, i1 = bucket_lo[b], bucket_hi[b]
                n = i1 - i0
                if n == 0:
                    continue
                pos = g0 * P
                gflat[pos : pos + n] = s_s[i0:i1].astype(np.int16)
                dflat[pos : pos + n] = d_s[i0:i1] % chunk - supers[sp]["col0"]
                wflat[pos : pos + n] = w_s[i0:i1]
        per_core.append(dict(
            gidx=np.tile(gflat.reshape(-1, 16).T, (8, 1)),
            dstloc=dflat.reshape(-1, P).T.astype(np.float32).copy(),
            wv=wflat.reshape(-1, P).T.copy(),
        ))

    layer = dict(ggroups=ggroups, spgroups=spgroups, total_slots=total_slots)
    return layer, per_core


def prep(edge_index, n_nodes, n_cores):
    src = np.asarray(edge_index[0], dtype=np.int64)
    dst = np.asarray(edge_index[1], dtype=np.int64)
    deg = np.bincount(dst, minlength=n_nodes) + 1
    w_e = (1.0 / np.sqrt(deg[src] * deg[dst])).astype(np.float32)

    assert n_nodes % n_cores == 0
    chunk = n_nodes // n_cores
    NT = math.ceil(chunk / P)
    NS = math.ceil(NT / TPS)
    DW = TPS * P

    supers = []
    for sp in range(NS):
        t0 = sp * TPS
        tiles = [(t, min(P, chunk - t * P)) for t in range(t0, min(t0 + TPS, NT))]
        supers.append(dict(sp=sp, tiles=tiles, col0=t0 * P))

    # layer 1: table = x rows (node ids), quarters of 32768
    NQ1 = math.ceil(n_nodes / QS)
    q1 = src // QS
    i1 = src - q1 * QS
    L1, pc1 = _bucketize(src, dst, w_e, q1, i1, n_cores, NS, NQ1, chunk, supers)
    L1["qbounds"] = [("X", q * QS, min(n_nodes, (q + 1) * QS)) for q in range(NQ1)]

    # layer 2: table = exchanged t rows, split into A (local row < SA) and B
    SA = SPLIT_SP * DW
    SB = chunk - SA
    rows_a = n_cores * SA
    rows_b = n_cores * SB
    c_of = src // chunk
    r_of = src % chunk
    in_a = r_of < SA
    row2 = np.where(in_a, c_of * SA + r_of, c_of * SB + (r_of - SA))
    nqa = math.ceil(rows_a / QS)
    nqb = math.ceil(rows_b / QS)
    q2 = np.where(in_a, row2 // QS, nqa + row2 // QS)
    i2 = row2 % QS
    NQ2 = nqa + nqb
    L2, pc2 = _bucketize(src, dst, w_e, q2, i2, n_cores, NS, NQ2, chunk, supers)
    L2["qbounds"] = [("A", q * QS, min(rows_a, (q + 1) * QS)) for q in range(nqa)] + [
        ("B", q * QS, min(rows_b, (q + 1) * QS)) for q in range(nqb)
    ]

    m = Meta()
    m.n_nodes = n_nodes
    m.n_cores = n_cores
    m.chunk = chunk
    m.NT = NT
    m.NS = NS
    m.dwidth = DW
    m.SA = SA
    m.SB = SB
    m.supers = supers
    m.L = [L1, L2]

    per_core = []
    for k in range(n_cores):
        gidx = np.concatenate([pc1[k]["gidx"], pc2[k]["gidx"]], axis=1)
        dstloc = np.concatenate([pc1[k]["dstloc"], pc2[k]["dstloc"]], axis=1)
        wv = np.concatenate([pc1[k]["wv"], pc2[k]["wv"]], axis=1)
        dn = np.ones(NT * P, dtype=np.float32)
        dn[:chunk] = deg[k * chunk : (k + 1) * chunk]
        degn = dn.reshape(NT, P).T.copy()
        per_core.append(dict(gidx=gidx, dstloc=dstloc, wv=wv, degn=degn))
    m.slots1 = L1["total_slots"]
    m.total_slots = L1["total_slots"] + L2["total_slots"]
    return m, per_core


# ---------------------------------------------------------------- kernel build

def build(m: Meta):
    nc = bacc.Bacc(trn_type="TRN2", num_devices=m.n_cores, target_bir_lowering=False)
    chunk, NT, NS, DW = m.chunk, m.NT, m.NS, m.dwidth

    xb_d = nc.dram_tensor("xb", [m.n_nodes, P], BF16, kind="ExternalInput")
    xown_d = nc.dram_tensor("xown", [chunk, P], BF16, kind="ExternalInput")
    w1_d = nc.dram_tensor("w1b", [F_IN, H1], BF16, kind="ExternalInput")
    w2a_d = nc.dram_tensor("w2ab", [P, H2], BF16, kind="ExternalInput")
    w2b_d = nc.dram_tensor("w2bb", [H1 - P, H2], BF16, kind="ExternalInput")
    wl_d = nc.dram_tensor("wlb", [H2, N_CLS], BF16, kind="ExternalInput")
    blx_d = nc.dram_tensor("blx4b", [1, N_CLS], BF16, kind="ExternalInput")
    b1a_d = nc.dram_tensor("b1a", [P, 1], F32, kind="ExternalInput")
    b1b_d = nc.dram_tensor("b1b", [H1 - P, 1], F32, kind="ExternalInput")
    b2_d = nc.dram_tensor("b2", [H2, 1], F32, kind="ExternalInput")
    gidx_d = nc.dram_tensor("gidx", [P, m.total_slots * 8], I16, kind="ExternalInput")
    dst_d = nc.dram_tensor("dstloc", [P, m.total_slots], F32, kind="ExternalInput")
    wv_d = nc.dram_tensor("wv", [P, m.total_slots], F32, kind="ExternalInput")
    degn_d = nc.dram_tensor("degn", [P, m.NT], F32, kind="ExternalInput")
    out_d = nc.dram_tensor("out", [chunk, N_CLS], F32, kind="ExternalOutput")

    tchA_d = nc.dram_tensor("tchA", [m.SA, P], BF16, kind="Internal")
    tchB_d = nc.dram_tensor("tchB", [m.SB, P], BF16, kind="Internal")
    tfA_d = nc.dram_tensor(
        "tfA", [m.n_cores * m.SA, P], BF16, kind="Internal", addr_space="Shared"
    )
    tfB_d = nc.dram_tensor(
        "tfB", [m.n_cores * m.SB, P], BF16, kind="Internal", addr_space="Shared"
    )

    def own_rows_l1(r0, r1):
        return xown_d[r0:r1, :]

    def own_rows_l2(r0, r1):
        if r1 <= m.SA:
            return tchA_d[r0:r1, :]
        assert r0 >= m.SA
        return tchB_d[r0 - m.SA : r1 - m.SA, :]

    from contextlib import ExitStack

    with tile.TileContext(nc) as tc, ExitStack() as ctx:
        cpool = ctx.enter_context(tc.tile_pool(name="consts", bufs=1))
        mpool = ctx.enter_context(tc.tile_pool(name="msg", bufs=2))
        spool = ctx.enter_context(tc.tile_pool(name="onehot", bufs=8))
        wkpool = ctx.enter_context(tc.tile_pool(name="work", bufs=3))
        softpool = ctx.enter_context(tc.tile_pool(name="soft", bufs=1))
        scat_pp = ctx.enter_context(tc.tile_pool(name="scat", bufs=2, space="PSUM"))
        y1a_pp = ctx.enter_context(tc.tile_pool(name="y1aps", bufs=2, space="PSUM"))
        y1b_pp = ctx.enter_context(tc.tile_pool(name="y1bps", bufs=1, space="PSUM"))
        t_pp = ctx.enter_context(tc.tile_pool(name="tps", bufs=1, space="PSUM"))
        log_pp = ctx.enter_context(tc.tile_pool(name="logps", bufs=1, space="PSUM"))

        # ---- constants / resident tiles
        w1_s = cpool.tile([F_IN, H1], BF16)
        w2a_s = cpool.tile([P, H2], BF16)
        w2b_s = cpool.tile([H1 - P, H2], BF16)
        wl_s = cpool.tile([H2, N_CLS], BF16)
        blx_s = cpool.tile([1, N_CLS], BF16)
        ones_s = cpool.tile([1, P], BF16)
        b1a_s = cpool.tile([P, 1], F32)
        b1b_s = cpool.tile([H1 - P, 1], F32)
        b2_s = cpool.tile([H2, 1], F32)
        gidx_s = cpool.tile([P, m.total_slots * 8], I16)
        dst_s = cpool.tile([P, m.total_slots], F32)
        wv_s = cpool.tile([P, m.total_slots], F32)
        negw_s = cpool.tile([P, m.total_slots], F32)
        iota_i = cpool.tile([P, DW], I32)
        iota_f = cpool.tile([P, DW], F32)
        ident_f = cpool.tile([P, P], F32)
        identw_f = cpool.tile([P, TPS, DW], F32)
        identw_b = cpool.tile([P, TPS, DW], BF16)
        degn_s = cpool.tile([P, m.NT], F32)
        dinvn_s = cpool.tile([P, m.NT], F32)
        lgall_s = cpool.tile([P, NT, N_CLS], F32)

        nc.sync.dma_start(w1_s[:], w1_d[:])
        nc.sync.dma_start(w2a_s[:], w2a_d[:])
        nc.sync.dma_start(w2b_s[:], w2b_d[:])
        nc.sync.dma_start(wl_s[:], wl_d[:])
        nc.sync.dma_start(blx_s[:], blx_d[:])
        nc.sync.dma_start(b1a_s[:], b1a_d[:])
        nc.sync.dma_start(b1b_s[:], b1b_d[:])
        nc.sync.dma_start(b2_s[:], b2_d[:])
        nc.sync.dma_start(gidx_s[:], gidx_d[:])
        nc.sync.dma_start(dst_s[:], dst_d[:])
        nc.sync.dma_start(wv_s[:], wv_d[:])
        nc.sync.dma_start(degn_s[:], degn_d[:])

        nc.vector.reciprocal(dinvn_s[:], degn_s[:])
        nc.vector.tensor_scalar_mul(negw_s[:], wv_s[:], -1.0)
        make_identity(nc, ident_f[:])
        nc.vector.memset(identw_f[:], 0)
        for ti in range(TPS):
            nc.vector.tensor_copy(identw_f[:, ti, ti * P : (ti + 1) * P], ident_f[:])
        nc.scalar.activation(
            identw_b[:], identw_f[:], mybir.ActivationFunctionType.Copy
        )
        nc.gpsimd.iota(iota_i[:], [[1, DW]], channel_multiplier=0)
        nc.vector.tensor_copy(iota_f[:], iota_i[:])
        nc.vector.memset(ones_s[:], 1.0)

        Prelu = mybir.ActivationFunctionType.Prelu
        Copy = mybir.ActivationFunctionType.Copy

        def exchange_a():
            if m.n_cores > 1:
                nc.gpsimd.collective_compute(
                    "AllGather", mybir.AluOpType.bypass,
                    replica_groups=[list(range(m.n_cores))],
                    ins=[tchA_d[:]], outs=[tfA_d[:]],
                )
            else:
                nc.sync.dma_start(tfA_d[:], tchA_d[:])

        def exchange_b():
            if m.n_cores > 1:
                nc.gpsimd.collective_compute(
                    "AllGather", mybir.AluOpType.bypass,
                    replica_groups=[list(range(m.n_cores))],
                    ins=[tchB_d[:]], outs=[tfB_d[:]],
                )
            else:
                nc.sync.dma_start(tfB_d[:], tchB_d[:])

        def layer(L, slot_off, table_aps, feat, epilogue, own_rows, post_gg=None):
            spgroups = L["spgroups"]
            for ggi, gg in enumerate(L["ggroups"]):
                msgs = {}
                for (q, g0, gn) in gg["qgroups"]:
                    ga = g0 + slot_off
                    mt = mpool.tile([P, gn, P], BF16, tag=f"msg{q % 4}")
                    nc.gpsimd.dma_gather(
                        out_ap=mt[:],
                        in_ap=table_aps[q],
                        idxs_ap=gidx_s[:, ga * 8 : (ga + gn) * 8],
                        num_idxs=gn * P,
                        num_idxs_reg=gn * P,
                        elem_size=P,
                        single_packet=False,
                    )
                    msgs[q] = (mt, g0)
                if post_gg is not None:
                    post_gg(ggi)
                for sp in gg["sps"]:
                    spm = m.supers[sp]
                    ntl = len(spm["tiles"])
                    ncols = ntl * P
                    scat = scat_pp.tile([P, DW], F32, tag="scat")
                    xt4 = wkpool.tile([P, TPS, P], BF16, tag="xt4")
                    nfull = sum(1 for (_, rows) in spm["tiles"] if rows == P)
                    t0 = spm["tiles"][0][0]
                    if nfull:
                        nc.sync.dma_start(
                            xt4[:, 0:nfull, :],
                            own_rows(t0 * P, (t0 + nfull) * P).rearrange(
                                "(a b) c -> b a c", b=P
                            ),
                        )
                    if nfull < ntl:
                        lt, lrows = spm["tiles"][nfull]
                        nc.sync.dma_start(
                            xt4[:lrows, nfull, :], own_rows(lt * P, lt * P + lrows)
                        )
                    for ti, (t, rows) in enumerate(spm["tiles"]):
                        dwt = spool.tile([P, DW], BF16, tag="S")
                        nc.scalar.activation(
                            dwt[:, :ncols], identw_b[:, ti, :ncols], Copy,
                            scale=dinvn_s[:, t : t + 1],
                        )
                        nc.tensor.matmul(
                            out=scat[:feat, :ncols],
                            lhsT=xt4[:rows, ti, :feat],
                            rhs=dwt[:rows, :ncols],
                            start=(ti == 0),
                            stop=False,
                        )
                    groups = spgroups[sp]
                    for gi, (q, g0, nsl) in enumerate(groups):
                        mt, mg0 = msgs[q]
                        for si in range(nsl):
                            g = g0 + si
                            ga = g + slot_off
                            # one-hot S[e,c] = w[e]*(c == d[e]); work is split
                            # between DVE (f32-in/bf16-out stt — the shape
                            # that avoids the shared DVE/GpSimd port pair)
                            # and the otherwise-idle ACT engine via
                            # sq=(c-d)^2 then Relu(w - w*sq).
                            S = spool.tile([P, DW], BF16, tag="S")
                            if g % 2 == 0:
                                nc.vector.scalar_tensor_tensor(
                                    out=S[:, :ncols],
                                    in0=iota_f[:, :ncols],
                                    scalar=dst_s[:, ga : ga + 1],
                                    in1=wv_s[:, ga : ga + 1].to_broadcast(
                                        [P, ncols]
                                    ),
                                    op0=mybir.AluOpType.is_equal,
                                    op1=mybir.AluOpType.mult,
                                )
                            else:
                                sq = spool.tile([P, DW], F32, tag="sq")
                                nc.scalar.activation(
                                    sq[:, :ncols], iota_f[:, :ncols],
                                    mybir.ActivationFunctionType.Square,
                                    bias=dst_s[:, ga : ga + 1], scale=-1.0,
                                )
                                nc.scalar.activation(
                                    S[:, :ncols], sq[:, :ncols],
                                    mybir.ActivationFunctionType.Relu,
                                    bias=wv_s[:, ga : ga + 1],
                                    scale=negw_s[:, ga : ga + 1],
                                )
                            nc.tensor.matmul(
                                out=scat[:feat, :ncols],
                                lhsT=mt[:, g - mg0, :feat],
                                rhs=S[:, :ncols],
                                start=False,
                                stop=(gi == len(groups) - 1 and si == nsl - 1),
                            )
                    epilogue(spm, scat)

        def write_t(t0row, nrows, t_sb, part_rows=None):
            """DMA t_sb [[128, k, 128]] to the A/B-split tchunk rows."""
            segs = []  # (dst tensor, dst row0, tile index range)
            if t0row < m.SA:
                segs.append((tchA_d, t0row, 0, min(nrows, m.SA - t0row)))
            if t0row + nrows > m.SA:
                b0 = max(t0row, m.SA)
                segs.append((tchB_d, b0 - m.SA, (b0 - t0row) // P, t0row + nrows - b0))
            for (dst, r0, ti0, rn) in segs:
                ntf = rn // P
                if ntf:
                    nc.sync.dma_start(
                        dst[r0 : r0 + ntf * P, :].rearrange("(a b) c -> b a c", b=P),
                        t_sb[:, ti0 : ti0 + ntf, :],
                    )
                rem = rn - ntf * P
                if rem:
                    nc.sync.dma_start(
                        dst[r0 + ntf * P : r0 + rn, :], t_sb[:rem, ti0 + ntf, :]
                    )

        def l1_epilogue(spm, scat):
            ntl = len(spm["tiles"])
            ncols = ntl * P
            h1b = wkpool.tile([P, DW], BF16, tag="h1b")
            nc.scalar.activation(h1b[:, :ncols], scat[:, :ncols], Copy)
            y1aps = y1a_pp.tile([P, DW], F32, tag="y1aps")
            y1bps = y1b_pp.tile([H1 - P, DW], F32, tag="y1bps")
            nc.tensor.matmul(
                out=y1aps[:, :ncols], lhsT=w1_s[:, 0:P], rhs=h1b[:, :ncols],
                start=True, stop=True,
            )
            nc.tensor.matmul(
                out=y1bps[:, :ncols], lhsT=w1_s[:, P:H1], rhs=h1b[:, :ncols],
                start=True, stop=True,
            )
            y1ab = wkpool.tile([P, DW], BF16, tag="y1ab")
            y1bb = wkpool.tile([H1 - P, DW], BF16, tag="y1bb")
            nc.scalar.activation(
                y1ab[:, :ncols], y1aps[:, :ncols], Prelu,
                bias=b1a_s[:, 0:1], scale=1.0, alpha=NEG_SLOPE,
            )
            nc.scalar.activation(
                y1bb[:, :ncols], y1bps[:, :ncols], Prelu,
                bias=b1b_s[:, 0:1], scale=1.0, alpha=NEG_SLOPE,
            )
            tps = t_pp.tile([P, TPS, H2], F32, tag="tps")
            for ti, (t, rows) in enumerate(spm["tiles"]):
                nc.tensor.matmul(
                    out=tps[:, ti, :], lhsT=y1ab[:, ti * P : (ti + 1) * P],
                    rhs=w2a_s[:], start=True, stop=False,
                )
                nc.tensor.matmul(
                    out=tps[:, ti, :], lhsT=y1bb[:, ti * P : (ti + 1) * P],
                    rhs=w2b_s[:], start=False, stop=True,
                )
            t_sb = wkpool.tile([P, TPS, P], BF16, tag="t_sb")
            nc.scalar.activation(t_sb[:, 0:ntl, 0:H2], tps[:, 0:ntl, :], Copy)
            t0 = spm["tiles"][0][0]
            nrows = sum(rows for (_, rows) in spm["tiles"])
            write_t(t0 * P, nrows, t_sb)

        def l2_epilogue(spm, scat):
            ntl = len(spm["tiles"])
            ncols = ntl * P
            y2b = wkpool.tile([H2, DW], BF16, tag="y2b")
            nc.scalar.activation(
                y2b[:, :ncols], scat[:H2, :ncols], Prelu,
                bias=b2_s[:, 0:1], scale=1.0, alpha=NEG_SLOPE,
            )
            lg = log_pp.tile([P, TPS, N_CLS], F32, tag="lg")
            for ti, (t, rows) in enumerate(spm["tiles"]):
                nc.tensor.matmul(
                    out=lg[:, ti, :], lhsT=y2b[:, ti * P : (ti + 1) * P],
                    rhs=wl_s[:], start=True, stop=False,
                )
                nc.tensor.matmul(
                    out=lg[:, ti, :], lhsT=ones_s[:],
                    rhs=blx_s[:], start=False, stop=True,
                )
            t0 = spm["tiles"][0][0]
            nc.scalar.activation(lgall_s[:, t0 : t0 + ntl, :], lg[:, 0:ntl, :], Copy)

        # ---- layer 1 (aggregate raw x, transform to t); AG_A issued mid-layer
        x_q = [xb_d[lo:hi, :] for (_, lo, hi) in m.L[0]["qbounds"]]

        def l1_post_gg(ggi):
            if ggi == AG_A_AT_GG:
                exchange_a()

        with nc.named_scope("layer1"):
            layer(m.L[0], 0, x_q, F_IN, l1_epilogue, own_rows_l1,
                  post_gg=l1_post_gg if len(m.L[0]["ggroups"]) > AG_A_AT_GG + 1
                  else None)
            if len(m.L[0]["ggroups"]) <= AG_A_AT_GG + 1:
                exchange_a()

        with nc.named_scope("exchange"):
            exchange_b()

        # ---- layer 2 + batched log_softmax
        t_q = []
        for (sp_, lo, hi) in m.L[1]["qbounds"]:
            t_q.append((tfA_d if sp_ == "A" else tfB_d)[lo:hi, :])
        with nc.named_scope("layer2"):
            layer(m.L[1], m.slots1, t_q, H2, l2_epilogue, own_rows_l2)

            Exp = mybir.ActivationFunctionType.Exp
            Ln = mybir.ActivationFunctionType.Ln
            negm = softpool.tile([P, NT, 1], F32, tag="negm")
            nc.vector.tensor_reduce(
                negm[:], lgall_s[:], mybir.AxisListType.X,
                mybir.AluOpType.max, negate=True,
            )
            # in-place: lgall becomes (logits - max); final result reuses ex
            nc.vector.tensor_tensor(
                out=lgall_s[:], in0=lgall_s[:],
                in1=negm[:].to_broadcast([P, NT, N_CLS]),
                op=mybir.AluOpType.add,
            )
            ex = softpool.tile([P, NT, N_CLS], F32, tag="ex")
            nc.scalar.activation(ex[:], lgall_s[:], Exp)
            ssum = softpool.tile([P, NT, 1], F32, tag="ssum")
            nc.vector.tensor_reduce(
                ssum[:], ex[:], mybir.AxisListType.X, mybir.AluOpType.add
            )
            lns = softpool.tile([P, NT, 1], F32, tag="lns")
            nc.scalar.activation(lns[:], ssum[:], Ln)
            osb = ex
            nc.vector.tensor_tensor(
                out=osb[:], in0=lgall_s[:],
                in1=lns[:].to_broadcast([P, NT, N_CLS]),
                op=mybir.AluOpType.subtract,
            )
            nfull_t = chunk // P
            nc.sync.dma_start(
                out_d[0 : nfull_t * P, :].rearrange("(a b) c -> b a c", b=P),
                osb[:, 0:nfull_t, :],
            )
            rem = chunk - nfull_t * P
            if rem:
                nc.sync.dma_start(
                    out_d[nfull_t * P : chunk, :], osb[:rem, nfull_t, :]
                )

    nc.compile()
    return nc


# ---------------------------------------------------------------- entry point

N_NODES = 100000
N_EDGES = 800000
N_CORES = 8

TRACE = False
LAST_EXEC_NS = None
LAST_RESULTS = None


def kernel(x, W1, b1, W2, b2, Wl, bl, edge_index):
    """Full-input GCN kernel: shards across 8 NeuronCores internally."""
    global LAST_EXEC_NS, LAST_RESULTS
    import ml_dtypes
    from concourse import bass_utils

    bf = ml_dtypes.bfloat16
    x = np.ascontiguousarray(np.asarray(x, dtype=np.float32))
    W1 = np.asarray(W1, dtype=np.float32)
    b1 = np.asarray(b1, dtype=np.float32).reshape(-1, 1)
    W2 = np.asarray(W2, dtype=np.float32)
    b2 = np.asarray(b2, dtype=np.float32).reshape(-1, 1)
    Wl = np.asarray(Wl, dtype=np.float32)
    bl = np.asarray(bl, dtype=np.float32).reshape(1, -1)
    edge_index = np.asarray(edge_index)

    n_nodes = x.shape[0]
    meta, per_core = prep(edge_index, n_nodes, n_cores=N_CORES)
    nc = build(meta)

    chunk = n_nodes // N_CORES
    xb = x.astype(bf)
    shared = dict(
        xb=xb,
        w1b=W1.astype(bf),
        w2ab=W2[:P].astype(bf),
        w2bb=W2[P:].astype(bf),
        wlb=Wl.astype(bf),
        blx4b=bl.astype(bf),
        b1a=b1[:P],
        b1b=b1[P:],
        b2=b2,
    )
    in_maps = [
        {**shared, "gidx": pc["gidx"], "dstloc": pc["dstloc"],
         "wv": pc["wv"], "degn": pc["degn"],
         "xown": xb[k * chunk : (k + 1) * chunk]}
        for k, pc in enumerate(per_core)
    ]
    res = bass_utils.run_bass_kernel_spmd(
        nc, in_maps, core_ids=list(range(N_CORES)), trace=TRACE
    )
    LAST_EXEC_NS = res.exec_time_ns
    LAST_RESULTS = res
    return np.concatenate([r["out"] for r in res.results], axis=0)


# revision 53
# speedup vs baseline: 1.2056x; 1.0387x over previous
"""2-layer GCN (PyG GCNConv semantics) as a Bass/Tile kernel for TRN2.

Math (per GCNConv layer, self-loops added, deg from dst in-degree + 1):
  out[d] = b + sum_{e: dst[e]=d} w[e] * t[src[e]]      with w[e] = rsqrt(deg[src]*deg[dst])
  where t = x        (layer 1: aggregate first, then @W1 — W commutes with aggregation)
        t = y1 @ W2  (layer 2: transform first)
  self-loop appears as an ordinary edge (i,i) with w = 1/deg[i].

Device mapping per core (nodes chunked across cores, edges bucketed by dst super):
  - supers of DW=512 dst columns; edges bucketed per (super, quarter) where a
    quarter is a <=32768-row slice of the gather table (int16 index limit),
    each bucket padded to a multiple of 128 (sizes are max over cores: one
    NEFF runs SPMD on all 8 cores with per-core index/weight data).
  - dma_gather (SWDGE) fetches 256B bf16 rows; calls are grouped over GSUP
    supers x 1 quarter (the Q7 desc-gen rate of ~8ns/row is the floor).
  - scatter-add via one-hot matmul: S[e, d] = w[e] * (dst_local[e] == d),
    built on DVE with f32 inputs and bf16 output (the one shape that stays
    off the shared DVE/GpSimd SBUF port pair — other dtype combos engage
    2-port perf mode and starve the Q7 descriptor generator);
    psum[f, d] += Msg^T @ S. Self-loops via dinv-scaled identity matmuls.
  - layer-1 t output is split at super SPLIT_SP: the first part is
    AllGathered mid-layer-1 so the exchange overlaps compute, the rest at
    the end; layer 2 gathers from the two exchanged tables.
  - log_softmax is batched over all tiles at the end (one Exp + one Ln).
"""

import math
import sys

import numpy as np

sys.path.insert(0, "/opt/trn_rl_repo")

import concourse.bass as bass
import concourse.bacc as bacc
import concourse.mybir as mybir
import concourse.tile as tile
from concourse.masks import make_identity

F32 = mybir.dt.float32
BF16 = mybir.dt.bfloat16
I16 = mybir.dt.int16
I32 = mybir.dt.int32

P = 128
QS = 32768  # int16-indexable rows per gather table slice
NEG_SLOPE = 0.01
TPS = 4        # dst tiles per super (DW = TPS*128)
GSUP = 3         # supers per grouped gather call
TAIL_SINGLES = 4 # final supers gathered as single-super groups (drain fast)
SPLIT_SP = 13    # supers 0..SPLIT_SP-1 exchange early (rows < SPLIT_SP*512)
AG_A_AT_GG = 5   # issue the early AllGather after this gather group's calls

F_IN, H1, H2, N_CLS = 128, 180, 120, 16


class Meta:
    pass


# ---------------------------------------------------------------- host prep

def _bucketize(src, dst, w_e, q_of, idx_of, n_cores, NS, NQ, chunk, supers):
    """Bucket edges by (core, super, quarter); lay out slot stream grouped
    for gather calls (gg -> q -> sp). Returns layer struct + per-core data."""
    core_of = dst // chunk
    sup_of = (dst % chunk) // (TPS * P)

    counts = np.zeros((n_cores, NS, NQ), dtype=np.int64)
    np.add.at(counts, (core_of, sup_of, q_of), 1)
    mx = counts.max(axis=0)
    slots_sq = ((mx + P - 1) // P).astype(np.int64)

    # group sizes: GSUP-wide groups, but finish each layer with single-super
    # groups so the end-of-layer consumption lag (bounded by the msg buffer
    # depth in groups) drains quickly after the last gather call
    sizes = []
    rem = NS
    while rem > TAIL_SINGLES:
        take = min(GSUP, rem - TAIL_SINGLES)
        sizes.append(take)
        rem -= take
    sizes.extend([1] * rem)
    slot0 = np.zeros((NS, NQ), dtype=np.int64)
    ggroups = []
    off = 0
    sp_next = 0
    for sz in sizes:
        sps = list(range(sp_next, sp_next + sz))
        sp_next += sz
        qg = []
        for q in range(NQ):
            g0 = off
            for sp in sps:
                slot0[sp, q] = off
                off += int(slots_sq[sp, q])
            if off > g0:
                qg.append((q, g0, off - g0))
        ggroups.append(dict(sps=sps, qgroups=qg))
    total_slots = off

    spgroups = []
    for sp in range(NS):
        spgroups.append([
            (q, int(slot0[sp, q]), int(slots_sq[sp, q]))
            for q in range(NQ)
            if slots_sq[sp, q] > 0
        ])

    order = np.lexsort((src, q_of, sup_of, core_of))
    s_s = idx_of[order]
    d_s = dst[order]
    w_s = w_e[order]
    keys = ((core_of * NS + sup_of) * NQ + q_of)[order]
    bucket_lo = np.searchsorted(keys, np.arange(n_cores * NS * NQ), side="left")
    bucket_hi = np.searchsorted(keys, np.arange(n_cores * NS * NQ), side="right")

    import ml_dtypes

    bf = ml_dtypes.bfloat16
    per_core = []
    for k in range(n_cores):
        gflat = np.zeros(total_slots * P, dtype=np.int16)
        dflat = np.full(total_slots * P, 999.0, dtype=np.float64)
        wflat = np.zeros(total_slots * P, dtype=np.float32)
        for sp in range(NS):
            for (q, g0, nsl) in spgroups[sp]:
                b = (k * NS + sp) * NQ + q
                i0, i1 = bucket_lo[b], bucket_hi[b]
                n = i1 - i0
                if n == 0:
                    continue
                pos = g0 * P
                gflat[pos : pos + n] = s_s[i0:i1].astype(np.int16)
                dflat[pos : pos + n] = d_s[i0:i1] % chunk - supers[sp]["col0"]
                wflat[pos : pos + n] = w_s[i0:i1]
        per_core.append(dict(
            gidx=np.tile(gflat.reshape(-1, 16).T, (8, 1)),
            dstloc=dflat.reshape(-1, P).T.astype(np.float32).copy(),
            wv=wflat.reshape(-1, P).T.copy(),
        ))

    layer = dict(ggroups=ggroups, spgroups=spgroups, total_slots=total_slots)
    return layer, per_core


def prep(edge_index, n_nodes, n_cores):
    src = np.asarray(edge_index[0], dtype=np.int64)
    dst = np.asarray(edge_index[1], dtype=np.int64)
    deg = np.bincount(dst, minlength=n_nodes) + 1
    w_e = (1.0 / np.sqrt(deg[src] * deg[dst])).astype(np.float32)

    assert n_nodes % n_cores == 0
    chunk = n_nodes // n_cores
    NT = math.ceil(chunk / P)
    NS = math.ceil(NT / TPS)
    DW = TPS * P

    supers = []
    for sp in range(NS):
        t0 = sp * TPS
        tiles = [(t, min(P, chunk - t * P)) for t in range(t0, min(t0 + TPS, NT))]
        supers.append(dict(sp=sp, tiles=tiles, col0=t0 * P))

    # layer 1: table = x rows (node ids), quarters of 32768
    NQ1 = math.ceil(n_nodes / QS)
    q1 = src // QS
    i1 = src - q1 * QS
    L1, pc1 = _bucketize(src, dst, w_e, q1, i1, n_cores, NS, NQ1, chunk, supers)
    L1["qbounds"] = [("X", q * QS, min(n_nodes, (q + 1) * QS)) for q in range(NQ1)]

    # layer 2: table = exchanged t rows, split into A (local row < SA) and B
    SA = SPLIT_SP * DW
    SB = chunk - SA
    rows_a = n_cores * SA
    rows_b = n_cores * SB
    c_of = src // chunk
    r_of = src % chunk
    in_a = r_of < SA
    row2 = np.where(in_a, c_of * SA + r_of, c_of * SB + (r_of - SA))
    nqa = math.ceil(rows_a / QS)
    nqb = math.ceil(rows_b / QS)
    q2 = np.where(in_a, row2 // QS, nqa + row2 // QS)
    i2 = row2 % QS
    NQ2 = nqa + nqb
    L2, pc2 = _bucketize(src, dst, w_e, q2, i2, n_cores, NS, NQ2, chunk, supers)
    L2["qbounds"] = [("A", q * QS, min(rows_a, (q + 1) * QS)) for q in range(nqa)] + [
        ("B", q * QS, min(rows_b, (q + 1) * QS)) for q in range(nqb)
    ]

    m = Meta()
    m.n_nodes = n_nodes
    m.n_cores = n_cores
    m.chunk = chunk
    m.NT = NT
    m.NS = NS
    m.dwidth = DW
    m.SA = SA
    m.SB = SB
    m.supers = supers
    m.L = [L1, L2]

    per_core = []
    for k in range(n_cores):
        gidx = np.concatenate([pc1[k]["gidx"], pc2[k]["gidx"]], axis=1)
        dstloc = np.concatenate([pc1[k]["dstloc"], pc2[k]["dstloc"]], axis=1)
        wv = np.concatenate([pc1[k]["wv"], pc2[k]["wv"]], axis=1)
        dn = np.ones(NT * P, dtype=np.float32)
        dn[:chunk] = deg[k * chunk : (k + 1) * chunk]
        degn = dn.reshape(NT, P).T.copy()
        per_core.append(dict(gidx=gidx, dstloc=dstloc, wv=wv, degn=degn))
    m.slots1 = L1["total_slots"]
    m.total_slots = L1["total_slots"] + L2["total_slots"]
    return m, per_core


# ---------------------------------------------------------------- kernel build

def build(m: Meta):
    nc = bacc.Bacc(trn_type="TRN2", num_devices=m.n_cores, target_bir_lowering=False)
    chunk, NT, NS, DW = m.chunk, m.NT, m.NS, m.dwidth

    xb_d = nc.dram_tensor("xb", [m.n_nodes, P], BF16, kind="ExternalInput")
    xown_d = nc.dram_tensor("xown", [chunk, P], BF16, kind="ExternalInput")
    w1_d = nc.dram_tensor("w1b", [F_IN, H1], BF16, kind="ExternalInput")
    w2a_d = nc.dram_tensor("w2ab", [P, H2], BF16, kind="ExternalInput")
    w2b_d = nc.dram_tensor("w2bb", [H1 - P, H2], BF16, kind="ExternalInput")
    wl_d = nc.dram_tensor("wlb", [H2, N_CLS], BF16, kind="ExternalInput")
    blx_d = nc.dram_tensor("blx4b", [1, N_CLS], BF16, kind="ExternalInput")
    b1a_d = nc.dram_tensor("b1a", [P, 1], F32, kind="ExternalInput")
    b1b_d = nc.dram_tensor("b1b", [H1 - P, 1], F32, kind="ExternalInput")
    b2_d = nc.dram_tensor("b2", [H2, 1], F32, kind="ExternalInput")
    gidx_d = nc.dram_tensor("gidx", [P, m.total_slots * 8], I16, kind="ExternalInput")
    dst_d = nc.dram_tensor("dstloc", [P, m.total_slots], F32, kind="ExternalInput")
    wv_d = nc.dram_tensor("wv", [P, m.total_slots], F32, kind="ExternalInput")
    degn_d = nc.dram_tensor("degn", [P, m.NT], F32, kind="ExternalInput")
    out_d = nc.dram_tensor("out", [chunk, N_CLS], F32, kind="ExternalOutput")

    tchA_d = nc.dram_tensor("tchA", [m.SA, P], BF16, kind="Internal")
    tchB_d = nc.dram_tensor("tchB", [m.SB, P], BF16, kind="Internal")
    tfA_d = nc.dram_tensor(
        "tfA", [m.n_cores * m.SA, P], BF16, kind="Internal", addr_space="Shared"
    )
    tfB_d = nc.dram_tensor(
        "tfB", [m.n_cores * m.SB, P], BF16, kind="Internal", addr_space="Shared"
    )

    def own_rows_l1(r0, r1):
        return xown_d[r0:r1, :]

    def own_rows_l2(r0, r1):
        if r1 <= m.SA:
            return tchA_d[r0:r1, :]
        assert r0 >= m.SA
        return tchB_d[r0 - m.SA : r1 - m.SA, :]

    from contextlib import ExitStack

    with tile.TileContext(nc) as tc, ExitStack() as ctx:
        cpool = ctx.enter_context(tc.tile_pool(name="consts", bufs=1))
        mpool = ctx.enter_context(tc.tile_pool(name="msg", bufs=2))
        spool = ctx.enter_context(tc.tile_pool(name="onehot", bufs=8))
        wkpool = ctx.enter_context(tc.tile_pool(name="work", bufs=3))
        softpool = ctx.enter_context(tc.tile_pool(name="soft", bufs=1))
        scat_pp = ctx.enter_context(tc.tile_pool(name="scat", bufs=2, space="PSUM"))
        y1a_pp = ctx.enter_context(tc.tile_pool(name="y1aps", bufs=2, space="PSUM"))
        y1b_pp = ctx.enter_context(tc.tile_pool(name="y1bps", bufs=1, space="PSUM"))
        t_pp = ctx.enter_context(tc.tile_pool(name="tps", bufs=1, space="PSUM"))
        log_pp = ctx.enter_context(tc.tile_pool(name="logps", bufs=1, space="PSUM"))

        # ---- constants / resident tiles
        w1_s = cpool.tile([F_IN, H1], BF16)
        w2a_s = cpool.tile([P, H2], BF16)
        w2b_s = cpool.tile([H1 - P, H2], BF16)
        wl_s = cpool.tile([H2, N_CLS], BF16)
        blx_s = cpool.tile([1, N_CLS], BF16)
        ones_s = cpool.tile([1, P], BF16)
        b1a_s = cpool.tile([P, 1], F32)
        b1b_s = cpool.tile([H1 - P, 1], F32)
        b2_s = cpool.tile([H2, 1], F32)
        gidx_s = cpool.tile([P, m.total_slots * 8], I16)
        dst_s = cpool.tile([P, m.total_slots], F32)
        wv_s = cpool.tile([P, m.total_slots], F32)
        negw_s = cpool.tile([P, m.total_slots], F32)
        iota_i = cpool.tile([P, DW], I32)
        iota_f = cpool.tile([P, DW], F32)
        ident_f = cpool.tile([P, P], F32)
        identw_f = cpool.tile([P, TPS, DW], F32)
        identw_b = cpool.tile([P, TPS, DW], BF16)
        degn_s = cpool.tile([P, m.NT], F32)
        dinvn_s = cpool.tile([P, m.NT], F32)
        lgall_s = cpool.tile([P, NT, N_CLS], F32)

        # gidx first: the opening dma_gather only needs it (+ the table)
        nc.sync.dma_start(gidx_s[:], gidx_d[:])
        nc.sync.dma_start(dst_s[:], dst_d[:])
        nc.sync.dma_start(wv_s[:], wv_d[:])
        nc.sync.dma_start(degn_s[:], degn_d[:])
        nc.sync.dma_start(w1_s[:], w1_d[:])
        nc.sync.dma_start(w2a_s[:], w2a_d[:])
        nc.sync.dma_start(w2b_s[:], w2b_d[:])
        nc.sync.dma_start(wl_s[:], wl_d[:])
        nc.sync.dma_start(blx_s[:], blx_d[:])
        nc.sync.dma_start(b1a_s[:], b1a_d[:])
        nc.sync.dma_start(b1b_s[:], b1b_d[:])
        nc.sync.dma_start(b2_s[:], b2_d[:])

        nc.vector.reciprocal(dinvn_s[:], degn_s[:])
        nc.vector.tensor_scalar_mul(negw_s[:], wv_s[:], -1.0)
        make_identity(nc, ident_f[:])
        nc.vector.memset(identw_f[:], 0)
        for ti in range(TPS):
            nc.vector.tensor_copy(identw_f[:, ti, ti * P : (ti + 1) * P], ident_f[:])
        nc.scalar.activation(
            identw_b[:], identw_f[:], mybir.ActivationFunctionType.Copy
        )
        nc.gpsimd.iota(iota_i[:], [[1, DW]], channel_multiplier=0)
        nc.vector.tensor_copy(iota_f[:], iota_i[:])
        nc.vector.memset(ones_s[:], 1.0)

        Prelu = mybir.ActivationFunctionType.Prelu
        Copy = mybir.ActivationFunctionType.Copy

        def exchange_a():
            if m.n_cores > 1:
                nc.gpsimd.collective_compute(
                    "AllGather", mybir.AluOpType.bypass,
                    replica_groups=[list(range(m.n_cores))],
                    ins=[tchA_d[:]], outs=[tfA_d[:]],
                )
            else:
                nc.sync.dma_start(tfA_d[:], tchA_d[:])

        def exchange_b():
            if m.n_cores > 1:
                nc.gpsimd.collective_compute(
                    "AllGather", mybir.AluOpType.bypass,
                    replica_groups=[list(range(m.n_cores))],
                    ins=[tchB_d[:]], outs=[tfB_d[:]],
                )
            else:
                nc.sync.dma_start(tfB_d[:], tchB_d[:])

        def layer(L, slot_off, table_aps, feat, epilogue, own_rows, post_gg=None):
            spgroups = L["spgroups"]
            for ggi, gg in enumerate(L["ggroups"]):
                msgs = {}
                for (q, g0, gn) in gg["qgroups"]:
                    ga = g0 + slot_off
                    mt = mpool.tile([P, gn, P], BF16, tag=f"msg{q % 4}")
                    nc.gpsimd.dma_gather(
                        out_ap=mt[:],
                        in_ap=table_aps[q],
                        idxs_ap=gidx_s[:, ga * 8 : (ga + gn) * 8],
                        num_idxs=gn * P,
                        num_idxs_reg=gn * P,
                        elem_size=P,
                        single_packet=False,
                    )
                    msgs[q] = (mt, g0)
                if post_gg is not None:
                    post_gg(ggi)
                for sp in gg["sps"]:
                    spm = m.supers[sp]
                    ntl = len(spm["tiles"])
                    ncols = ntl * P
                    scat = scat_pp.tile([P, DW], F32, tag="scat")
                    xt4 = wkpool.tile([P, TPS, P], BF16, tag="xt4")
                    nfull = sum(1 for (_, rows) in spm["tiles"] if rows == P)
                    t0 = spm["tiles"][0][0]
                    if nfull:
                        nc.sync.dma_start(
                            xt4[:, 0:nfull, :],
                            own_rows(t0 * P, (t0 + nfull) * P).rearrange(
                                "(a b) c -> b a c", b=P
                            ),
                        )
                    if nfull < ntl:
                        lt, lrows = spm["tiles"][nfull]
                        nc.sync.dma_start(
                            xt4[:lrows, nfull, :], own_rows(lt * P, lt * P + lrows)
                        )
                    for ti, (t, rows) in enumerate(spm["tiles"]):
                        dwt = spool.tile([P, DW], BF16, tag="S")
                        nc.scalar.activation(
                            dwt[:, :ncols], identw_b[:, ti, :ncols], Copy,
                            scale=dinvn_s[:, t : t + 1],
                        )
                        nc.tensor.matmul(
                            out=scat[:feat, :ncols],
                            lhsT=xt4[:rows, ti, :feat],
                            rhs=dwt[:rows, :ncols],
                            start=(ti == 0),
                            stop=False,
                        )
                    groups = spgroups[sp]
                    for gi, (q, g0, nsl) in enumerate(groups):
                        mt, mg0 = msgs[q]
                        for si in range(nsl):
                            g = g0 + si
                            ga = g + slot_off
                            # one-hot S[e,c] = w[e]*(c == d[e]); work is split
                            # between DVE (f32-in/bf16-out stt — the shape
                            # that avoids the shared DVE/GpSimd port pair)
                            # and the otherwise-idle ACT engine via
                            # sq=(c-d)^2 then Relu(w - w*sq).
                            S = spool.tile([P, DW], BF16, tag="S")
                            if g % 2 == 0:
                                nc.vector.scalar_tensor_tensor(
                                    out=S[:, :ncols],
                                    in0=iota_f[:, :ncols],
                                    scalar=dst_s[:, ga : ga + 1],
                                    in1=wv_s[:, ga : ga + 1].to_broadcast(
                                        [P, ncols]
                                    ),
                                    op0=mybir.AluOpType.is_equal,
                                    op1=mybir.AluOpType.mult,
                                )
                            else:
                                sq = spool.tile([P, DW], F32, tag="sq")
                                nc.scalar.activation(
                                    sq[:, :ncols], iota_f[:, :ncols],
                                    mybir.ActivationFunctionType.Square,
                                    bias=dst_s[:, ga : ga + 1], scale=-1.0,
                                )
                                nc.scalar.activation(
                                    S[:, :ncols], sq[:, :ncols],
                                    mybir.ActivationFunctionType.Relu,
                                    bias=wv_s[:, ga : ga + 1],
                                    scale=negw_s[:, ga : ga + 1],
                                )
                            nc.tensor.matmul(
                                out=scat[:feat, :ncols],
                                lhsT=mt[:, g - mg0, :feat],
                                rhs=S[:, :ncols],
                                start=False,
                                stop=(gi == len(groups) - 1 and si == nsl - 1),
                            )
                    epilogue(spm, scat)

        def write_t(t0row, nrows, t_sb, part_rows=None):
            """DMA t_sb [[128, k, 128]] to the A/B-split tchunk rows."""
            segs = []  # (dst tensor, dst row0, tile index range)
            if t0row < m.SA:
                segs.append((tchA_d, t0row, 0, min(nrows, m.SA - t0row)))
            if t0row + nrows > m.SA:
                b0 = max(t0row, m.SA)
                segs.append((tchB_d, b0 - m.SA, (b0 - t0row) // P, t0row + nrows - b0))
            for (dst, r0, ti0, rn) in segs:
                ntf = rn // P
                if ntf:
                    nc.sync.dma_start(
                        dst[r0 : r0 + ntf * P, :].rearrange("(a b) c -> b a c", b=P),
                        t_sb[:, ti0 : ti0 + ntf, :],
                    )
                rem = rn - ntf * P
                if rem:
                    nc.sync.dma_start(
                        dst[r0 + ntf * P : r0 + rn, :], t_sb[:rem, ti0 + ntf, :]
                    )

        def l1_epilogue(spm, scat):
            ntl = len(spm["tiles"])
            ncols = ntl * P
            h1b = wkpool.tile([P, DW], BF16, tag="h1b")
            nc.scalar.activation(h1b[:, :ncols], scat[:, :ncols], Copy)
            y1aps = y1a_pp.tile([P, DW], F32, tag="y1aps")
            y1bps = y1b_pp.tile([H1 - P, DW], F32, tag="y1bps")
            nc.tensor.matmul(
                out=y1aps[:, :ncols], lhsT=w1_s[:, 0:P], rhs=h1b[:, :ncols],
                start=True, stop=True,
            )
            nc.tensor.matmul(
                out=y1bps[:, :ncols], lhsT=w1_s[:, P:H1], rhs=h1b[:, :ncols],
                start=True, stop=True,
            )
            y1ab = wkpool.tile([P, DW], BF16, tag="y1ab")
            y1bb = wkpool.tile([H1 - P, DW], BF16, tag="y1bb")
            nc.scalar.activation(
                y1ab[:, :ncols], y1aps[:, :ncols], Prelu,
                bias=b1a_s[:, 0:1], scale=1.0, alpha=NEG_SLOPE,
            )
            nc.scalar.activation(
                y1bb[:, :ncols], y1bps[:, :ncols], Prelu,
                bias=b1b_s[:, 0:1], scale=1.0, alpha=NEG_SLOPE,
            )
            tps = t_pp.tile([P, TPS, H2], F32, tag="tps")
            for ti, (t, rows) in enumerate(spm["tiles"]):
                nc.tensor.matmul(
                    out=tps[:, ti, :], lhsT=y1ab[:, ti * P : (ti + 1) * P],
                    rhs=w2a_s[:], start=True, stop=False,
                )
                nc.tensor.matmul(
                    out=tps[:, ti, :], lhsT=y1bb[:, ti * P : (ti + 1) * P],
                    rhs=w2b_s[:], start=False, stop=True,
                )
            t_sb = wkpool.tile([P, TPS, P], BF16, tag="t_sb")
            nc.scalar.activation(t_sb[:, 0:ntl, 0:H2], tps[:, 0:ntl, :], Copy)
            t0 = spm["tiles"][0][0]
            nrows = sum(rows for (_, rows) in spm["tiles"])
            write_t(t0 * P, nrows, t_sb)

        def l2_epilogue(spm, scat):
            ntl = len(spm["tiles"])
            ncols = ntl * P
            y2b = wkpool.tile([H2, DW], BF16, tag="y2b")
            nc.scalar.activation(
                y2b[:, :ncols], scat[:H2, :ncols], Prelu,
                bias=b2_s[:, 0:1], scale=1.0, alpha=NEG_SLOPE,
            )
            lg = log_pp.tile([P, TPS, N_CLS], F32, tag="lg")
            for ti, (t, rows) in enumerate(spm["tiles"]):
                nc.tensor.matmul(
                    out=lg[:, ti, :], lhsT=y2b[:, ti * P : (ti + 1) * P],
                    rhs=wl_s[:], start=True, stop=False,
                )
                nc.tensor.matmul(
                    out=lg[:, ti, :], lhsT=ones_s[:],
                    rhs=blx_s[:], start=False, stop=True,
                )
            t0 = spm["tiles"][0][0]
            nc.scalar.activation(lgall_s[:, t0 : t0 + ntl, :], lg[:, 0:ntl, :], Copy)

        # ---- layer 1 (aggregate raw x, transform to t); AG_A issued mid-layer
        x_q = [xb_d[lo:hi, :] for (_, lo, hi) in m.L[0]["qbounds"]]

        def l1_post_gg(ggi):
            if ggi == AG_A_AT_GG:
                exchange_a()

        with nc.named_scope("layer1"):
            layer(m.L[0], 0, x_q, F_IN, l1_epilogue, own_rows_l1,
                  post_gg=l1_post_gg if len(m.L[0]["ggroups"]) > AG_A_AT_GG + 1
                  else None)
            if len(m.L[0]["ggroups"]) <= AG_A_AT_GG + 1:
                exchange_a()

        with nc.named_scope("exchange"):
            exchange_b()

        # ---- layer 2 + batched log_softmax
        t_q = []
        for (sp_, lo, hi) in m.L[1]["qbounds"]:
            t_q.append((tfA_d if sp_ == "A" else tfB_d)[lo:hi, :])
        with nc.named_scope("layer2"):
            layer(m.L[1], m.slots1, t_q, H2, l2_epilogue, own_rows_l2)

            Exp = mybir.ActivationFunctionType.Exp
            Ln = mybir.ActivationFunctionType.Ln
            negm = softpool.tile([P, NT, 1], F32, tag="negm")
            nc.vector.tensor_reduce(
                negm[:], lgall_s[:], mybir.AxisListType.X,
                mybir.AluOpType.max, negate=True,
            )
            # in-place: lgall becomes (logits - max); final result reuses ex
            nc.vector.tensor_tensor(
                out=lgall_s[:], in0=lgall_s[:],
                in1=negm[:].to_broadcast([P, NT, N_CLS]),
                op=mybir.AluOpType.add,
            )
            ex = softpool.tile([P, NT, N_CLS], F32, tag="ex")
            nc.scalar.activation(ex[:], lgall_s[:], Exp)
            ssum = softpool.tile([P, NT, 1], F32, tag="ssum")
            nc.vector.tensor_reduce(
                ssum[:], ex[:], mybir.AxisListType.X, mybir.AluOpType.add
            )
            lns = softpool.tile([P, NT, 1], F32, tag="lns")
            nc.scalar.activation(lns[:], ssum[:], Ln)
            osb = ex
            nc.vector.tensor_tensor(
                out=osb[:], in0=lgall_s[:],
                in1=lns[:].to_broadcast([P, NT, N_CLS]),
                op=mybir.AluOpType.subtract,
            )
            nfull_t = chunk // P
            nc.sync.dma_start(
                out_d[0 : nfull_t * P, :].rearrange("(a b) c -> b a c", b=P),
                osb[:, 0:nfull_t, :],
            )
            rem = chunk - nfull_t * P
            if rem:
                nc.sync.dma_start(
                    out_d[nfull_t * P : chunk, :], osb[:rem, nfull_t, :]
                )

    nc.compile()
    return nc


# ---------------------------------------------------------------- entry point

N_NODES = 100000
N_EDGES = 800000
N_CORES = 8

TRACE = False
LAST_EXEC_NS = None
LAST_RESULTS = None


def kernel(x, W1, b1, W2, b2, Wl, bl, edge_index):
    """Full-input GCN kernel: shards across 8 NeuronCores internally."""
    global LAST_EXEC_NS, LAST_RESULTS
    import ml_dtypes
    from concourse import bass_utils

    bf = ml_dtypes.bfloat16
    x = np.ascontiguousarray(np.asarray(x, dtype=np.float32))
    W1 = np.asarray(W1, dtype=np.float32)
    b1 = np.asarray(b1, dtype=np.float32).reshape(-1, 1)
    W2 = np.asarray(W2, dtype=np.float32)
    b2 = np.asarray(b2, dtype=np.float32).reshape(-1, 1)
    Wl = np.asarray(Wl, dtype=np.float32)
    bl = np.asarray(bl, dtype=np.float32).reshape(1, -1)
    edge_index = np.asarray(edge_index)

    n_nodes = x.shape[0]
    meta, per_core = prep(edge_index, n_nodes, n_cores=N_CORES)
    nc = build(meta)

    chunk = n_nodes // N_CORES
    xb = x.astype(bf)
    shared = dict(
        xb=xb,
        w1b=W1.astype(bf),
        w2ab=W2[:P].astype(bf),
        w2bb=W2[P:].astype(bf),
        wlb=Wl.astype(bf),
        blx4b=bl.astype(bf),
        b1a=b1[:P],
        b1b=b1[P:],
        b2=b2,
    )
    in_maps = [
        {**shared, "gidx": pc["gidx"], "dstloc": pc["dstloc"],
         "wv": pc["wv"], "degn": pc["degn"],
         "xown": xb[k * chunk : (k + 1) * chunk]}
        for k, pc in enumerate(per_core)
    ]
    res = bass_utils.run_bass_kernel_spmd(
        nc, in_maps, core_ids=list(range(N_CORES)), trace=TRACE
    )
    LAST_EXEC_NS = res.exec_time_ns
    LAST_RESULTS = res
    return np.concatenate([r["out"] for r in res.results], axis=0)
